# revision 16
# baseline (speedup 1.0000x reference)
"""AttentionalPooler Trainium2 kernel.

Full inputs -> full outputs; internally data-parallel over batch across 8
NeuronCores (b=8, one batch element per core).

Per-core math (one batch element, all in fp32):
  xk  = LN(x)                      [4096, 1024]
  q   = (LN(query) @ Wq) * scale   [256, 1024]   (identical on every core)
  kT  = Wk'^T @ xk^T               [1024, 4096]  (K stored transposed)
  V   = xk @ Wv'                   [4096, 1024]  (row-major, +ones col/head)
  S^T = kT_h^T-slices @ qT_h       [4096, 256] per head  (j on partitions)
  E   = exp(S^T)  (no max subtraction; |S| <= ~7 so fp32-safe)
  [O^T_h; den_h] = [V_h | 1]^T @ E  accumulated over j   [65, 256]
  out = sum_h (O_h / den_h) @ Wout_h                     [256, 1024]

Schedule: quarters of 1024 keys are software-pipelined -- LN/transpose/
K/V-projection for quarter q+1 (written into double-buffered kT/V tiles)
is interleaved into the attention instruction stream of quarter q, so the
exp (ACT) latency bubbles are filled with projection matmuls and the PE
never idles long enough for the HAM clock gate to re-throttle.  Softmax
normalization and the out-projection are woven into the last quarter,
one head-pair at a time, leaving only a tiny serial tail.
"""

import os
import sys
import types

for _p in ("/root/.axon_site", "/root/.axon_site/_ro/trn_rl_repo", "/opt/trn_rl_repo"):
    if os.path.isdir(_p) and _p not in sys.path:
        sys.path.append(_p)

# The image's antenv package lacks axon_hooks; shim it with the ctypes-based
# NTFF hook from trn_agent_boot so trace=True works under axon.
try:
    import antenv.axon_hooks  # noqa: F401
except ImportError:
    try:
        import trn_agent_boot.trn_boot as _tb

        _hook = _tb._ntff_profile_via_ctypes("/opt/axon/libaxon_pjrt.so")
    except Exception:
        _hook = None
    _m = types.ModuleType("antenv.axon_hooks")
    _m.get_axon_ntff_profile_hook = lambda: _hook
    sys.modules["antenv.axon_hooks"] = _m

import numpy as np

import concourse.bass as bass
import concourse.tile as tile
from concourse import mybir
from concourse.masks import make_identity

D = 1024          # model dim == ctx dim
NCTX = 4096       # keys per batch element
NQ = 256          # queries
H = 16            # heads
DH = 64           # head dim
NCORES = 8
EPS = 1e-5
QTR = 1024        # keys per pipelined quarter
SUP = 512         # projection super-tile (j)
NJJ = QTR // 128  # 128-key chunks per quarter

F32 = mybir.dt.float32
BF16 = mybir.dt.bfloat16

MM_DT = BF16


def _mm_np():
    if MM_DT == F32:
        return np.float32
    import ml_dtypes

    return ml_dtypes.bfloat16


def _patch_drain(max_waits=1):
    """This walrus build rejects >1 sync-wait on the SP Drain that Tile emits
    at kernel exit. Split the waits across a chain of drains."""

    def patched(self, tick_clock, wait_clock):
        from concourse.vector_clock import ScopedClock

        drain_inst = self.nc.sync.drain()
        wait_clock.add_sem_waits(
            drain_inst.ins, ScopedClock({None: tick_clock.global_clock})
        )
        si = drain_inst.ins.sync_info
        waits = list(si.on_wait or []) if si else []
        if len(waits) > max_waits:
            si.on_wait = waits[:max_waits]
            rest = waits[max_waits:]
            while rest:
                extra = self.nc.sync.drain()
                extra.ins.sync_info = mybir.SyncInfo(
                    on_wait=rest[:max_waits], on_update=[]
                )
                rest = rest[max_waits:]
        self.nc.all_engine_barrier()
        assert self.sems is not None
        popped = self.nc._tile_sem_poison_stack.pop()
        assert popped is self._sem_poison
        self.nc.clear_and_free_semaphores(list(self.sems.allocated().values()))
        self.nc.all_engine_barrier()

    tile.TileContext._drain_and_barrier = patched


_patch_drain()


def _split_sync_waits(nc, max_waits=1):
    """This walrus build rejects instructions carrying more than one sync
    wait. Hoist excess waits onto same-engine NoOps placed just before the
    owning instruction (engine queues are serial, so this is equivalent)."""
    for f in nc.m.functions:
        for bb in f.blocks:
            new_list = []
            changed = False
            for inst in bb.instructions:
                si = inst.sync_info
                waits = list(si.on_wait) if si and si.on_wait else []
                if len(waits) > max_waits:
                    changed = True
                    keep = waits[-max_waits:]
                    rest = waits[:-max_waits]
                    k = 0
                    while rest:
                        carrier = mybir.InstNoOp(
                            name=f"{inst.name}-w{k}", ins=[], outs=[]
                        )
                        carrier.engine = inst.engine
                        carrier.sync_info = mybir.SyncInfo(
                            on_wait=rest[:max_waits], on_update=[]
                        )
                        rest = rest[max_waits:]
                        k += 1
                        nc.register_instruction(carrier, overwrite=True)
                        new_list.append(carrier)
                    si.on_wait = keep
                new_list.append(inst)
            if changed:
                bb.instructions = new_list
    return nc


def build_program():
    nc = bass.Bass("TRN2", target_bir_lowering=False, debug=False)

    x = nc.dram_tensor("x", [NCTX, D], F32, kind="ExternalInput").ap()
    qry = nc.dram_tensor("qry", [NQ, D], F32, kind="ExternalInput").ap()
    wq = nc.dram_tensor("wq", [D, D], MM_DT, kind="ExternalInput").ap()
    wk = nc.dram_tensor("wk", [D, D], MM_DT, kind="ExternalInput").ap()
    wv = nc.dram_tensor("wv", [D, D], MM_DT, kind="ExternalInput").ap()
    wo = nc.dram_tensor("wo", [D, D], MM_DT, kind="ExternalInput").ap()
    bq = nc.dram_tensor("bq", [128, 8], F32, kind="ExternalInput").ap()
    bk = nc.dram_tensor("bk", [128, 8], F32, kind="ExternalInput").ap()
    bv = nc.dram_tensor("bv", [D], F32, kind="ExternalInput").ap()
    out = nc.dram_tensor("out", [NQ, D], F32, kind="ExternalOutput").ap()

    with tile.TileContext(nc) as tc:
        _build_body(nc, tc, x, qry, wq, wk, wv, wo, bq, bk, bv, out)
    _split_sync_waits(nc)
    return nc


class _Body:
    """Holds all tiles/pools; methods emit instruction groups."""

    def __init__(self, nc, tc, ctx, x, qry, wq, wk, wv, wo, bq, bk, bv, out):
        self.nc = nc
        self.tc = tc
        self.x, self.qry = x, qry
        self.wq, self.wk, self.wv, self.wo = wq, wk, wv, wo
        self.bq_d, self.bk_d, self.bv_d, self.out_d = bq, bk, bv, out

        ec = ctx.enter_context
        self.consts = ec(tc.tile_pool(name="consts", bufs=1))
        self.wpool = ec(tc.tile_pool(name="wpool", bufs=1))
        self.wstream = ec(tc.tile_pool(name="wstream", bufs=2))
        self.xpool = ec(tc.tile_pool(name="xpool", bufs=4))
        self.big = ec(tc.tile_pool(name="big", bufs=1))
        self.kvp = ec(tc.tile_pool(name="kvp", bufs=2))
        self.xkp = ec(tc.tile_pool(name="xkp", bufs=2))
        self.per = ec(tc.tile_pool(name="per", bufs=2))
        self.etp = ec(tc.tile_pool(name="etp", bufs=4))
        self.outp = ec(tc.tile_pool(name="outp", bufs=2))
        # PSUM budget (8 banks x 2KB):
        #   mm 2x2KB (proj evac; out-proj ic=0 chains live here in q3)
        #   st 2x2KB (sim S^T pairs; also the den-broadcast in q3)
        #   ot0/ot1 1x2KB each (attention accumulators: one bank per
        #   head so the two heads' accumulation groups never share a bank)
        #   tr 2x[128,2,128]bf16 (transposes)
        self.ps_mm = ec(tc.tile_pool(name="ps_mm", bufs=2, space="PSUM"))
        self.ps_st = ec(tc.tile_pool(name="ps_st", bufs=2, space="PSUM"))
        self.ps_ot = ec(tc.tile_pool(name="ps_ot", bufs=1, space="PSUM"))
        self.ps_tr = ec(tc.tile_pool(name="ps_tr", bufs=2, space="PSUM"))

        self.identb = self.consts.tile([128, 128], MM_DT, tag="identb")
        make_identity(nc, self.identb)
        self.eps_t = self.consts.tile([128, 1], F32, tag="eps")
        nc.vector.memset(self.eps_t, EPS)
        self.ones_t = self.consts.tile([128, DH], F32, tag="ones_t")
        nc.vector.memset(self.ones_t, 1.0)
        self.bq_sb = self.consts.tile([128, 8], F32, tag="bq")
        nc.gpsimd.dma_start(out=self.bq_sb, in_=self.bq_d)
        self.bk_sb = self.consts.tile([128, 8], F32, tag="bk")
        nc.gpsimd.dma_start(out=self.bk_sb, in_=self.bk_d)
        self.bv_rep = self.consts.tile([128, D], F32, tag="bvrep")
        bv_bcast = bass.AP(
            tensor=self.bv_d.tensor, offset=self.bv_d.offset,
            ap=[[0, 128]] + list(self.bv_d.ap),
        )
        nc.gpsimd.dma_start(out=self.bv_rep, in_=bv_bcast)

        self.qT = self.consts.tile([128, 8, NQ], MM_DT, tag="qT")
        self.wk_sb = self.wpool.tile([128, 8, D], MM_DT, tag="wk")
        self.wv_sb = self.wpool.tile([128, 8, D], MM_DT, tag="wv")
        self.wo_sb = self.wpool.tile([64, H, D], MM_DT, tag="wo")
        self.otacc = self.big.tile([65, H, NQ], F32, tag="ot")
        self.ot_n = self.big.tile([64, H, NQ], MM_DT, tag="otn")

        self.kv = {}     # quarter -> (kT_q, v_q) double-buffered tiles
        self.psf = None  # out-proj ic=0 PSUM chains (allocated at q3 start)

    # ---------- phase A (LN + transpose + K/V projection) ----------

    def phaseA_ops(self, q):
        """Closure list building kT/v for quarter q, finely sliced so it can
        be interleaved into the attention stream of quarter q-1."""
        nc = self.nc
        st = {}

        def alloc():
            kT = self.kvp.tile([128, 8, QTR], MM_DT, tag="kt")
            v = self.kvp.tile([128, NJJ, H * 65], MM_DT, tag="vq")
            self.kv[q] = (kT, v)
            ones = v.rearrange("p j (h c) -> p j h c", c=65)[:, :, :, 64:65]
            nc.vector.memset(ones, 1.0)

        def load(s):
            def f():
                xts = []
                for jt in range(SUP // 128):
                    j0 = q * QTR + s * SUP + jt * 128
                    xt = self.xpool.tile([128, D], F32, tag="xt")
                    nc.sync.dma_start(out=xt, in_=self.x[j0:j0 + 128, :])
                    xts.append(xt)
                st[("xt", s)] = xts
            return f

        def stats(s, t):
            def f():
                if ("mv", s) not in st:
                    st[("mv", s)] = self.per.tile([128, 4, 2], F32, tag="mv", name="mv_s")
                xt = st[("xt", s)][t]
                stt = self.per.tile([128, 2, nc.vector.BN_STATS_DIM], F32,
                                    tag="stats")
                for sg in range(2):
                    nc.vector.bn_stats(
                        out=stt[:, sg, :], in_=xt[:, sg * 512:(sg + 1) * 512]
                    )
                nc.vector.bn_aggr(out=st[("mv", s)][:, t, :], in_=stt)
            return f

        def rstd(s):
            def f():
                sig = self.per.tile([128, 4], F32, tag="sig")
                nc.scalar.activation(
                    out=sig, in_=st[("mv", s)][:, :, 1],
                    func=mybir.ActivationFunctionType.Sqrt,
                    bias=self.eps_t, scale=1.0,
                )
                r = self.per.tile([128, 4], F32, tag="rstd")
                nc.vector.reciprocal(out=r, in_=sig)
                st[("rstd", s)] = r
            return f

        def norm(s, t):
            def f():
                xnb = self.xpool.tile([128, D], MM_DT, tag="xnb")
                nc.vector.tensor_scalar(
                    out=xnb, in0=st[("xt", s)][t],
                    scalar1=st[("mv", s)][:, t, 0:1],
                    scalar2=st[("rstd", s)][:, t:t + 1],
                    op0=mybir.AluOpType.subtract, op1=mybir.AluOpType.mult,
                )
                st.setdefault(("xnb", s), {})[t] = xnb
            return f

        def tr(s, t):
            def f():
                if ("xkT", s) not in st:
                    st[("xkT", s)] = self.xkp.tile([128, 8, SUP], MM_DT,
                                                   tag="xkT", name="xkTs")
                xnb = st[("xnb", s)][t]
                xkT = st[("xkT", s)]
                for c in range(4):
                    ptr = self.ps_tr.tile([128, 2, 128], MM_DT, tag="tr")
                    for k in range(2):
                        dc = c * 2 + k
                        nc.tensor.transpose(
                            ptr[:, k, :], xnb[:, dc * 128:(dc + 1) * 128],
                            self.identb,
                        )
                    nc.vector.tensor_copy(
                        out=xkT[:, c * 2:c * 2 + 2, t * 128:(t + 1) * 128],
                        in_=ptr,
                    )
            return f

        def kproj(s, ec):
            def f():
                psk = self.ps_mm.tile([128, SUP], F32, tag="mm")
                for dc in range(8):
                    nc.tensor.matmul(
                        psk,
                        lhsT=self.wk_sb[:, dc, ec * 128:(ec + 1) * 128],
                        rhs=st[("xkT", s)][:, dc, :],
                        start=(dc == 0), stop=(dc == 7),
                    )
                nc.vector.tensor_scalar(
                    out=self.kv[q][0][:, ec, s * SUP:(s + 1) * SUP], in0=psk,
                    scalar1=self.bk_sb[:, ec:ec + 1], scalar2=None,
                    op0=mybir.AluOpType.add,
                )
            return f

        def vproj(s, jt, nt):
            def f():
                psv = self.ps_mm.tile([128, SUP], F32, tag="mm")
                for dc in range(8):
                    nc.tensor.matmul(
                        psv,
                        lhsT=st[("xkT", s)][:, dc, jt * 128:(jt + 1) * 128],
                        rhs=self.wv_sb[:, dc, nt * 512:(nt + 1) * 512],
                        start=(dc == 0), stop=(dc == 7),
                    )
                jj = s * (SUP // 128) + jt
                vdst = self.kv[q][1][
                    :, jj, nt * 8 * 65:(nt + 1) * 8 * 65
                ].rearrange("p (h c) -> p h c", c=65)[:, :, 0:64]
                nc.vector.tensor_add(
                    out=vdst,
                    in0=psv.rearrange("p (h c) -> p h c", c=64),
                    in1=self.bv_rep[:, nt * 512:(nt + 1) * 512].rearrange(
                        "p (h c) -> p h c", c=64
                    ),
                )
            return f

        ops = [alloc, load(0)]
        ops += [stats(0, t) for t in range(4)]
        ops.append(rstd(0))
        ops += [norm(0, t) for t in range(4)]
        ops += [tr(0, t) for t in range(4)]
        ops.append(load(1))
        ops += [stats(1, t) for t in range(4)]
        ops.append(rstd(1))
        ops += [norm(1, t) for t in range(4)]
        # super-0 projections with super-1 transposes spread among them
        mix = [kproj(0, e) for e in range(8)]
        mix += [vproj(0, jt, nt) for jt in range(4) for nt in range(2)]
        tr1 = [tr(1, t) for t in range(4)]
        for i, m in enumerate(mix):
            ops.append(m)
            if i % 4 == 3 and tr1:
                ops.append(tr1.pop(0))
        ops += tr1
        ops += [kproj(1, e) for e in range(8)]
        ops += [vproj(1, jt, nt) for jt in range(4) for nt in range(2)]
        return ops

    # ---------- attention ----------

    def attn_unit(self, q, hc, jjp, psos):
        """One double-chunk: 4 sim MMs, 2 exps, 4 PV MMs.  The row-tiled
        sim pair (rows 0-63 / 64-127) must land in DIFFERENT PSUM banks --
        concurrent row-tiles share a bank's write port otherwise."""
        nc = self.nc
        kT, v = self.kv[q]
        pstp0 = self.ps_st.tile([128, 2, NQ], F32, tag="st", name="pstp0")
        pstp1 = self.ps_st.tile([128, 2, NQ], F32, tag="st", name="pstp1")
        pstps = (pstp0, pstp1)
        for u in range(2):
            jj = jjp * 2 + u
            for par in range(2):
                pb = par * 64
                nc.tensor.matmul(
                    pstps[par][:, u, :],
                    lhsT=kT[pb:pb + 64, hc, jj * 128:(jj + 1) * 128],
                    rhs=self.qT[pb:pb + 64, hc, :],
                    start=True, stop=True,
                )
        ets = []
        for par in range(2):
            et = self.etp.tile([128, 2, NQ], MM_DT, tag="et", name="et")
            nc.scalar.activation(
                out=et, in_=pstps[par],
                func=mybir.ActivationFunctionType.Exp,
            )
            ets.append(et)
        for u in range(2):
            jj = jjp * 2 + u
            for par in range(2):
                h = hc * 2 + par
                nc.tensor.matmul(
                    psos[par],
                    lhsT=v[:, jj, h * 65:(h + 1) * 65],
                    rhs=ets[par][:, u, :],
                    start=(jj == 0), stop=(jj == NJJ - 1),
                )

    def np_recip(self, hc):
        """Stage 1 of head-pair normalize: 1/den in place (DVE, f32)."""
        self.nc.vector.reciprocal(
            out=self.otacc[64:65, hc * 2:hc * 2 + 2, :],
            in_=self.otacc[64:65, hc * 2:hc * 2 + 2, :],
        )

    def np_scale(self, hc):
        """Stage 2: broadcast 1/den down 64 partitions and scale O."""
        nc = self.nc
        psb = self.ps_st.tile([128, 2, NQ], F32, tag="st")
        for k in range(2):
            h = hc * 2 + k
            nc.tensor.matmul(
                psb[0:64, k, :], lhsT=self.ones_t[64:65, :],
                rhs=self.otacc[64:65, h, :],
                start=True, stop=True,
            )
            nc.vector.tensor_mul(
                out=self.ot_n[:, h, :], in0=self.otacc[0:64, h, :],
                in1=psb[0:64, k, :],
            )

    def np_oproj(self, hc):
        """Stage 3: ic=0 half of the out-projection for the head pair."""
        nc = self.nc
        for k in range(2):
            h = hc * 2 + k
            for ft in range(2):
                nc.tensor.matmul(
                    self.psf[ft],
                    lhsT=self.ot_n[:, h, 0:128],
                    rhs=self.wo_sb[:, h, ft * 512:(ft + 1) * 512],
                    start=(h == 0), stop=(h == 15),
                )

    def attention_ops(self, q):
        nc = self.nc
        ops = []
        st = {}

        def unit(hc, jjp):
            def f():
                if ("psos", hc) not in st:
                    st[("psos", hc)] = [
                        self.ps_ot.tile([65, NQ], F32, tag="ot0", name="psos0"),
                        self.ps_ot.tile([65, NQ], F32, tag="ot1", name="psos1"),
                    ]
                self.attn_unit(q, hc, jjp, st[("psos", hc)])
            return f

        def fin(hc):
            def f():
                psos = st.pop(("psos", hc))
                for par in range(2):
                    dst = self.otacc[:, hc * 2 + par, :]
                    if q == 0:
                        nc.vector.tensor_copy(out=dst, in_=psos[par])
                    else:
                        nc.vector.tensor_add(out=dst, in0=dst, in1=psos[par])
            return f

        if q == 3:
            def alloc_psf():
                self.psf = [
                    self.ps_mm.tile([128, 512], F32, tag="mm", name="psf")
                    for _ in range(2)
                ]
            ops.append(alloc_psf)
        for hc in range(8):
            for jjp in range(NJJ // 2):
                ops.append(unit(hc, jjp))
                # stagger normalize stages so the PE never waits on the
                # DVE reciprocal/scale (each stage trails by >= 2 units)
                if q == 3 and jjp == 2 and hc >= 1:
                    ops.append(lambda hc=hc: self.np_scale(hc - 1))
            ops.append(fin(hc))
            if q == 3:
                ops.append(lambda hc=hc: self.np_recip(hc))
                if hc >= 1:
                    ops.append(lambda hc=hc: self.np_oproj(hc - 1))
        if q == 3:
            ops.append(lambda: self.np_scale(7))
            ops.append(lambda: self.np_oproj(7))
        return ops

    # ---------- one-time pieces ----------

    def weights_dma(self):
        nc = self.nc
        wk_r = self.wk.rearrange("(c p) e -> p c e", p=128)
        wv_r = self.wv.rearrange("(c p) e -> p c e", p=128)
        for dc in range(8):
            nc.scalar.dma_start(out=self.wk_sb[:, dc, :], in_=wk_r[:, dc, :])
        for dc in range(8):
            nc.scalar.dma_start(out=self.wv_sb[:, dc, :], in_=wv_r[:, dc, :])

    def wo_dma(self):
        self.nc.scalar.dma_start(
            out=self.wo_sb, in_=self.wo.rearrange("(h p) f -> p h f", p=64)
        )

    def qproj_ops(self):
        nc = self.nc
        ops = []
        st = {}

        def load():
            qts = []
            for t in range(2):
                qt = self.xpool.tile([128, D], F32, tag="xt")
                nc.sync.dma_start(out=qt, in_=self.qry[t * 128:(t + 1) * 128, :])
                qts.append(qt)
            st["qts"] = qts
        ops.append(load)

        def ln_and_tr():
            qts = st["qts"]
            mv = self.per.tile([128, 2, 2], F32, tag="mv")
            for t in range(2):
                stt = self.per.tile([128, 2, nc.vector.BN_STATS_DIM], F32,
                                    tag="stats")
                for sg in range(2):
                    nc.vector.bn_stats(
                        out=stt[:, sg, :],
                        in_=qts[t][:, sg * 512:(sg + 1) * 512],
                    )
                nc.vector.bn_aggr(out=mv[:, t, :], in_=stt)
            sig = self.per.tile([128, 2], F32, tag="sig")
            nc.scalar.activation(
                out=sig, in_=mv[:, :, 1],
                func=mybir.ActivationFunctionType.Sqrt,
                bias=self.eps_t, scale=1.0,
            )
            r = self.per.tile([128, 2], F32, tag="rstd")
            nc.vector.reciprocal(out=r, in_=sig)
            qnT = self.xkp.tile([128, 8, SUP], MM_DT, tag="xkT")
            st["qnT"] = qnT
            for t in range(2):
                qnb = self.xpool.tile([128, D], MM_DT, tag="xnb")
                nc.vector.tensor_scalar(
                    out=qnb, in0=qts[t], scalar1=mv[:, t, 0:1],
                    scalar2=r[:, t:t + 1],
                    op0=mybir.AluOpType.subtract, op1=mybir.AluOpType.mult,
                )
                for c in range(4):
                    ptr = self.ps_tr.tile([128, 2, 128], MM_DT, tag="tr")
                    for k in range(2):
                        dc = c * 2 + k
                        nc.tensor.transpose(
                            ptr[:, k, :], qnb[:, dc * 128:(dc + 1) * 128],
                            self.identb,
                        )
                    nc.vector.tensor_copy(
                        out=qnT[:, c * 2:c * 2 + 2, t * 128:(t + 1) * 128],
                        in_=ptr,
                    )
        ops.append(ln_and_tr)

        wq_r = self.wq.rearrange("(c p) e -> p c e", p=128)

        def proj_ec(ec):
            def f():
                wq_t = self.wstream.tile([128, 8, 128], MM_DT, tag="wqs")
                nc.sync.dma_start(
                    out=wq_t, in_=wq_r[:, :, ec * 128:(ec + 1) * 128]
                )
                psq = self.ps_mm.tile([128, NQ], F32, tag="mm")
                for dc in range(8):
                    nc.tensor.matmul(
                        psq, lhsT=wq_t[:, dc, :], rhs=st["qnT"][:, dc, 0:NQ],
                        start=(dc == 0), stop=(dc == 7),
                    )
                nc.vector.tensor_scalar(
                    out=self.qT[:, ec, :], in0=psq,
                    scalar1=self.bq_sb[:, ec:ec + 1], scalar2=None,
                    op0=mybir.AluOpType.add,
                )
            return f
        for ec in range(8):
            ops.append(proj_ec(ec))
        return ops

    def dummy_out(self):
        nc = self.nc
        osb = self.outp.tile([128, D], F32, tag="outsb", name="osb")
        nc.vector.memset(osb, 0.0)
        nc.sync.dma_start(out=self.out_d[0:128, :], in_=osb)
        nc.sync.dma_start(out=self.out_d[128:256, :], in_=osb)

    def tail(self):
        """ic=0 evac + full ic=1 out-projection chain + store."""
        nc = self.nc
        osb = self.outp.tile([128, D], F32, tag="outsb", name="osb")
        for ft in range(2):
            nc.scalar.activation(
                out=osb[:, ft * 512:(ft + 1) * 512], in_=self.psf[ft],
                func=mybir.ActivationFunctionType.Copy,
            )
        nc.sync.dma_start(out=self.out_d[0:128, :], in_=osb)
        psf2 = [self.ps_mm.tile([128, 512], F32, tag="mm", name="psf2") for _ in range(2)]
        for h in range(16):
            for ft in range(2):
                nc.tensor.matmul(
                    psf2[ft],
                    lhsT=self.ot_n[:, h, 128:256],
                    rhs=self.wo_sb[:, h, ft * 512:(ft + 1) * 512],
                    start=(h == 0), stop=(h == 15),
                )
        osb2 = self.outp.tile([128, D], F32, tag="outsb", name="osb2")
        for ft in range(2):
            nc.scalar.activation(
                out=osb2[:, ft * 512:(ft + 1) * 512], in_=psf2[ft],
                func=mybir.ActivationFunctionType.Copy,
            )
        nc.sync.dma_start(out=self.out_d[128:256, :], in_=osb2)


def _interleave(primary, secondary):
    """Emit all of `primary` with `secondary` spread evenly among them."""
    ops = []
    if not primary:
        return list(secondary)
    ratio = len(secondary) / len(primary)
    acc = 0.0
    si = 0
    for p in primary:
        ops.append(p)
        acc += ratio
        while si < len(secondary) and acc >= 1.0 - 1e-9:
            ops.append(secondary[si])
            si += 1
            acc -= 1.0
    ops.extend(secondary[si:])
    return ops


def _build_body(nc, tc, x, qry, wq, wk, wv, wo, bq, bk, bv, out):
    import contextlib

    ctx = contextlib.ExitStack()
    with ctx:
        b = _Body(nc, tc, ctx, x, qry, wq, wk, wv, wo, bq, bk, bv, out)

        pa0 = b.phaseA_ops(0)
        qp = b.qproj_ops()
        # startup: q load + LN/transpose strictly first (their tiles sit at
        # the head of the shared rings), then quarter-0 LN/transposes with
        # the q-projection matmuls confined to before the super-1 transposes
        # (so the qnT ring slot is provably released in PE order)
        ops = [qp[0], qp[1], pa0[0], pa0[1], b.weights_dma]
        ops += _interleave(pa0[2:24], qp[2:] + [b.wo_dma])
        ops += pa0[24:]
        for o in ops:
            o()

        import os as _os
        seq = bool(int(_os.environ.get("KERNEL_NO_INTERLEAVE", "0")))
        stage = int(_os.environ.get("KERNEL_STAGE", "4"))
        if stage <= 1:
            b.dummy_out()
            return
        for q in range(4):
            attn = b.attention_ops(q)
            nxt = b.phaseA_ops(q + 1) if q < 3 else []
            if seq:
                for o in attn + nxt:
                    o()
                continue
            head, rest = attn[:4], attn[4:]
            for o in head:
                o()
            for o in _interleave(rest, nxt):
                o()
            if stage < 4 and stage <= q + 2:
                b.dummy_out()
                return

        b.tail()


_CACHED = None


def _get_program():
    global _CACHED
    if _CACHED is None:
        _CACHED = build_program()
    return _CACHED


def _prep_inputs(x, query, Wq, Wkv, Wout, ln_q_g, ln_q_b, ln_k_g, ln_k_b):
    scale = DH ** -0.5
    f32 = np.float32
    Wq = np.asarray(Wq, f32)
    Wkv = np.asarray(Wkv, f32)
    Wout = np.asarray(Wout, f32)
    wq_eff = (np.asarray(ln_q_g, f32)[:, None] * Wq * scale).astype(f32)
    bq_eff = (np.asarray(ln_q_b, f32) @ Wq * scale).astype(f32)
    wk_eff = (np.asarray(ln_k_g, f32)[:, None] * Wkv[:, :D]).astype(f32)
    bk_eff = (np.asarray(ln_k_b, f32) @ Wkv[:, :D]).astype(f32)
    wv_eff = (np.asarray(ln_k_g, f32)[:, None] * Wkv[:, D:]).astype(f32)
    bv_eff = (np.asarray(ln_k_b, f32) @ Wkv[:, D:]).astype(f32)
    mdt = _mm_np()
    shared = {
        "qry": np.ascontiguousarray(np.asarray(query, f32)),
        "wq": np.ascontiguousarray(wq_eff.astype(mdt)),
        "wk": np.ascontiguousarray(wk_eff.astype(mdt)),
        "wv": np.ascontiguousarray(wv_eff.astype(mdt)),
        "wo": np.ascontiguousarray(Wout.astype(mdt)),
        "bq": np.ascontiguousarray(bq_eff.reshape(8, 128).T),
        "bk": np.ascontiguousarray(bk_eff.reshape(8, 128).T),
        "bv": np.ascontiguousarray(bv_eff),
    }
    x = np.asarray(x, f32)
    in_maps = [
        dict(shared, x=np.ascontiguousarray(x[i])) for i in range(NCORES)
    ]
    return in_maps


def run(trace=False, **inputs):
    from concourse.bass_utils import run_bass_kernel_spmd

    nc = _get_program()
    in_maps = _prep_inputs(**inputs)
    res = run_bass_kernel_spmd(
        nc, in_maps, core_ids=list(range(NCORES)), trace=trace
    )
    out = np.stack([res.results[i]["out"] for i in range(NCORES)], axis=0)
    return out.astype(np.float32), res.exec_time_ns


def kernel(**inputs):
    out, _ = run(trace=False, **inputs)
    return out


# revision 19
# speedup vs baseline: 1.2096x; 1.2096x over previous
"""AttentionalPooler Trainium2 kernel.

Full inputs -> full outputs; internally data-parallel over batch across 8
NeuronCores (b=8, one batch element per core).

Per-core math (one batch element, all in fp32):
  xk  = LN(x)                      [4096, 1024]
  q   = (LN(query) @ Wq) * scale   [256, 1024]   (identical on every core)
  kT  = Wk'^T @ xk^T               [1024, 4096]  (K stored transposed)
  V   = xk @ Wv'                   [4096, 1024]  (row-major, +ones col/head)
  S^T = kT_h^T-slices @ qT_h       [4096, 256] per head  (j on partitions)
  E   = exp(S^T)  (no max subtraction; |S| <= ~7 so fp32-safe)
  [O^T_h; den_h] = [V_h | 1]^T @ E  accumulated over j   [65, 256]
  out = sum_h (O_h / den_h) @ Wout_h                     [256, 1024]

Schedule: quarters of 1024 keys are software-pipelined -- LN/transpose/
K/V-projection for quarter q+1 (written into double-buffered kT/V tiles)
is interleaved into the attention instruction stream of quarter q, so the
exp (ACT) latency bubbles are filled with projection matmuls and the PE
never idles long enough for the HAM clock gate to re-throttle.  Softmax
normalization and the out-projection are woven into the last quarter,
one head-pair at a time, leaving only a tiny serial tail.
"""

import os
import sys
import types

for _p in ("/root/.axon_site", "/root/.axon_site/_ro/trn_rl_repo", "/opt/trn_rl_repo"):
    if os.path.isdir(_p) and _p not in sys.path:
        sys.path.append(_p)

# The image's antenv package lacks axon_hooks; shim it with the ctypes-based
# NTFF hook from trn_agent_boot so trace=True works under axon.
try:
    import antenv.axon_hooks  # noqa: F401
except ImportError:
    try:
        import trn_agent_boot.trn_boot as _tb

        _hook = _tb._ntff_profile_via_ctypes("/opt/axon/libaxon_pjrt.so")
    except Exception:
        _hook = None
    _m = types.ModuleType("antenv.axon_hooks")
    _m.get_axon_ntff_profile_hook = lambda: _hook
    sys.modules["antenv.axon_hooks"] = _m

import numpy as np

import concourse.bass as bass
import concourse.tile as tile
from concourse import mybir
from concourse.masks import make_identity

D = 1024          # model dim == ctx dim
NCTX = 4096       # keys per batch element
NQ = 256          # queries
H = 16            # heads
DH = 64           # head dim
NCORES = 8
EPS = 1e-5
QTR = 1024        # keys per pipelined quarter
SUP = 512         # projection super-tile (j)
NJJ = QTR // 128  # 128-key chunks per quarter

F32 = mybir.dt.float32
BF16 = mybir.dt.bfloat16

MM_DT = BF16


def _mm_np():
    if MM_DT == F32:
        return np.float32
    import ml_dtypes

    return ml_dtypes.bfloat16


def _patch_drain(max_waits=1):
    """This walrus build rejects >1 sync-wait on the SP Drain that Tile emits
    at kernel exit. Split the waits across a chain of drains."""

    def patched(self, tick_clock, wait_clock):
        from concourse.vector_clock import ScopedClock

        drain_inst = self.nc.sync.drain()
        wait_clock.add_sem_waits(
            drain_inst.ins, ScopedClock({None: tick_clock.global_clock})
        )
        si = drain_inst.ins.sync_info
        waits = list(si.on_wait or []) if si else []
        if len(waits) > max_waits:
            si.on_wait = waits[:max_waits]
            rest = waits[max_waits:]
            while rest:
                extra = self.nc.sync.drain()
                extra.ins.sync_info = mybir.SyncInfo(
                    on_wait=rest[:max_waits], on_update=[]
                )
                rest = rest[max_waits:]
        self.nc.all_engine_barrier()
        assert self.sems is not None
        popped = self.nc._tile_sem_poison_stack.pop()
        assert popped is self._sem_poison
        self.nc.clear_and_free_semaphores(list(self.sems.allocated().values()))
        self.nc.all_engine_barrier()

    tile.TileContext._drain_and_barrier = patched


_patch_drain()


def _split_sync_waits(nc, max_waits=1):
    """This walrus build rejects instructions carrying more than one sync
    wait. Hoist excess waits onto same-engine NoOps placed just before the
    owning instruction (engine queues are serial, so this is equivalent)."""
    for f in nc.m.functions:
        for bb in f.blocks:
            new_list = []
            changed = False
            for inst in bb.instructions:
                si = inst.sync_info
                waits = list(si.on_wait) if si and si.on_wait else []
                if len(waits) > max_waits:
                    changed = True
                    keep = waits[-max_waits:]
                    rest = waits[:-max_waits]
                    k = 0
                    while rest:
                        carrier = mybir.InstNoOp(
                            name=f"{inst.name}-w{k}", ins=[], outs=[]
                        )
                        carrier.engine = inst.engine
                        carrier.sync_info = mybir.SyncInfo(
                            on_wait=rest[:max_waits], on_update=[]
                        )
                        rest = rest[max_waits:]
                        k += 1
                        nc.register_instruction(carrier, overwrite=True)
                        new_list.append(carrier)
                    si.on_wait = keep
                new_list.append(inst)
            if changed:
                bb.instructions = new_list
    return nc


def build_program():
    nc = bass.Bass("TRN2", target_bir_lowering=False, debug=False)

    x = nc.dram_tensor("x", [NCTX, D], F32, kind="ExternalInput").ap()
    qry = nc.dram_tensor("qry", [NQ, D], F32, kind="ExternalInput").ap()
    wq = nc.dram_tensor("wq", [D, D], MM_DT, kind="ExternalInput").ap()
    wk = nc.dram_tensor("wk", [D, D], MM_DT, kind="ExternalInput").ap()
    wv = nc.dram_tensor("wv", [D, D], MM_DT, kind="ExternalInput").ap()
    wo = nc.dram_tensor("wo", [D, D], MM_DT, kind="ExternalInput").ap()
    bq = nc.dram_tensor("bq", [128, 8], F32, kind="ExternalInput").ap()
    bk = nc.dram_tensor("bk", [128, 8], F32, kind="ExternalInput").ap()
    bv = nc.dram_tensor("bv", [D], F32, kind="ExternalInput").ap()
    out = nc.dram_tensor("out", [NQ, D], F32, kind="ExternalOutput").ap()

    with tile.TileContext(nc) as tc:
        _build_body(nc, tc, x, qry, wq, wk, wv, wo, bq, bk, bv, out)
    _split_sync_waits(nc)
    return nc


class _Body:
    """Holds all tiles/pools; methods emit instruction groups."""

    def __init__(self, nc, tc, ctx, x, qry, wq, wk, wv, wo, bq, bk, bv, out):
        self.nc = nc
        self.tc = tc
        self.x, self.qry = x, qry
        self.wq, self.wk, self.wv, self.wo = wq, wk, wv, wo
        self.bq_d, self.bk_d, self.bv_d, self.out_d = bq, bk, bv, out

        ec = ctx.enter_context
        self.consts = ec(tc.tile_pool(name="consts", bufs=1))
        self.wpool = ec(tc.tile_pool(name="wpool", bufs=1))
        self.wstream = ec(tc.tile_pool(name="wstream", bufs=2))
        self.xpool = ec(tc.tile_pool(name="xpool", bufs=4))
        self.big = ec(tc.tile_pool(name="big", bufs=1))
        self.kvp = ec(tc.tile_pool(name="kvp", bufs=2))
        self.xkp = ec(tc.tile_pool(name="xkp", bufs=2))
        self.per = ec(tc.tile_pool(name="per", bufs=2))
        self.etp = ec(tc.tile_pool(name="etp", bufs=4))
        self.outp = ec(tc.tile_pool(name="outp", bufs=2))
        # PSUM budget (8 banks x 2KB):
        #   mm 2x2KB (proj evac; out-proj ic=0 chains live here in q3)
        #   st 2x2KB (sim S^T pairs; also the den-broadcast in q3)
        #   ot0/ot1 1x2KB each (attention accumulators: one bank per
        #   head so the two heads' accumulation groups never share a bank)
        #   tr 2x[128,2,128]bf16 (transposes)
        self.ps_mm = ec(tc.tile_pool(name="ps_mm", bufs=2, space="PSUM"))
        self.ps_st = ec(tc.tile_pool(name="ps_st", bufs=2, space="PSUM"))
        self.ps_ot = ec(tc.tile_pool(name="ps_ot", bufs=1, space="PSUM"))
        self.ps_tr = ec(tc.tile_pool(name="ps_tr", bufs=2, space="PSUM"))

        self.identb = self.consts.tile([128, 128], MM_DT, tag="identb")
        make_identity(nc, self.identb)
        self.eps_t = self.consts.tile([128, 1], F32, tag="eps")
        nc.vector.memset(self.eps_t, EPS)
        self.ones_t = self.consts.tile([128, DH], F32, tag="ones_t")
        nc.vector.memset(self.ones_t, 1.0)
        self.bq_sb = self.consts.tile([128, 8], F32, tag="bq")
        nc.gpsimd.dma_start(out=self.bq_sb, in_=self.bq_d)
        self.bk_sb = self.consts.tile([128, 8], F32, tag="bk")
        nc.gpsimd.dma_start(out=self.bk_sb, in_=self.bk_d)
        self.bv_rep = self.consts.tile([128, D], F32, tag="bvrep")
        bv_bcast = bass.AP(
            tensor=self.bv_d.tensor, offset=self.bv_d.offset,
            ap=[[0, 128]] + list(self.bv_d.ap),
        )
        nc.gpsimd.dma_start(out=self.bv_rep, in_=bv_bcast)

        self.qT = self.consts.tile([128, 8, NQ], MM_DT, tag="qT")
        self.wk_sb = self.wpool.tile([128, 8, D], MM_DT, tag="wk")
        self.wv_sb = self.wpool.tile([128, 8, D], MM_DT, tag="wv")
        self.wo_sb = self.wpool.tile([64, H, D], MM_DT, tag="wo")
        self.otacc = self.big.tile([65, H, NQ], F32, tag="ot")
        self.ot_n = self.big.tile([64, H, NQ], MM_DT, tag="otn")

        self.kv = {}     # quarter -> (kT_q, v_q) double-buffered tiles
        self.psf = None  # out-proj ic=0 PSUM chains (allocated at q3 start)

    # ---------- phase A (LN + transpose + K/V projection) ----------

    def phaseA_ops(self, q):
        """Closure list building kT/v for quarter q, finely sliced so it can
        be interleaved into the attention stream of quarter q-1."""
        nc = self.nc
        st = {}

        def alloc():
            kT = self.kvp.tile([128, 8, QTR], MM_DT, tag="kt")
            v = self.kvp.tile([128, NJJ, H * 65], MM_DT, tag="vq")
            self.kv[q] = (kT, v)
            ones = v.rearrange("p j (h c) -> p j h c", c=65)[:, :, :, 64:65]
            nc.vector.memset(ones, 1.0)

        def load(s):
            def f():
                xts = []
                for jt in range(SUP // 128):
                    j0 = q * QTR + s * SUP + jt * 128
                    xt = self.xpool.tile([128, D], F32, tag="xt")
                    nc.sync.dma_start(out=xt, in_=self.x[j0:j0 + 128, :])
                    xts.append(xt)
                st[("xt", s)] = xts
            return f

        def stats(s, t):
            def f():
                if ("mv", s) not in st:
                    st[("mv", s)] = self.per.tile([128, 4, 2], F32, tag="mv", name="mv_s")
                xt = st[("xt", s)][t]
                stt = self.per.tile([128, 2, nc.vector.BN_STATS_DIM], F32,
                                    tag="stats")
                for sg in range(2):
                    nc.vector.bn_stats(
                        out=stt[:, sg, :], in_=xt[:, sg * 512:(sg + 1) * 512]
                    )
                nc.vector.bn_aggr(out=st[("mv", s)][:, t, :], in_=stt)
            return f

        def rstd(s):
            def f():
                sig = self.per.tile([128, 4], F32, tag="sig")
                nc.scalar.activation(
                    out=sig, in_=st[("mv", s)][:, :, 1],
                    func=mybir.ActivationFunctionType.Sqrt,
                    bias=self.eps_t, scale=1.0,
                )
                r = self.per.tile([128, 4], F32, tag="rstd")
                nc.vector.reciprocal(out=r, in_=sig)
                st[("rstd", s)] = r
            return f

        def norm(s, t):
            def f():
                xnb = self.xpool.tile([128, D], MM_DT, tag="xnb")
                nc.vector.tensor_scalar(
                    out=xnb, in0=st[("xt", s)][t],
                    scalar1=st[("mv", s)][:, t, 0:1],
                    scalar2=st[("rstd", s)][:, t:t + 1],
                    op0=mybir.AluOpType.subtract, op1=mybir.AluOpType.mult,
                )
                st.setdefault(("xnb", s), {})[t] = xnb
            return f

        def tr(s, t):
            def f():
                if ("xkT", s) not in st:
                    st[("xkT", s)] = self.xkp.tile([128, 8, SUP], MM_DT,
                                                   tag="xkT", name="xkTs")
                xnb = st[("xnb", s)][t]
                xkT = st[("xkT", s)]
                for c in range(4):
                    ptr = self.ps_tr.tile([128, 2, 128], MM_DT, tag="tr")
                    for k in range(2):
                        dc = c * 2 + k
                        nc.tensor.transpose(
                            ptr[:, k, :], xnb[:, dc * 128:(dc + 1) * 128],
                            self.identb,
                        )
                    nc.vector.tensor_copy(
                        out=xkT[:, c * 2:c * 2 + 2, t * 128:(t + 1) * 128],
                        in_=ptr,
                    )
            return f

        def kproj(s, ec):
            def f():
                psk = self.ps_mm.tile([128, SUP], F32, tag="mm")
                for dc in range(8):
                    nc.tensor.matmul(
                        psk,
                        lhsT=self.wk_sb[:, dc, ec * 128:(ec + 1) * 128],
                        rhs=st[("xkT", s)][:, dc, :],
                        start=(dc == 0), stop=(dc == 7),
                    )
                nc.vector.tensor_scalar(
                    out=self.kv[q][0][:, ec, s * SUP:(s + 1) * SUP], in0=psk,
                    scalar1=self.bk_sb[:, ec:ec + 1], scalar2=None,
                    op0=mybir.AluOpType.add,
                )
            return f

        def vproj(s, jt, nt):
            def f():
                psv = self.ps_mm.tile([128, SUP], F32, tag="mm")
                for dc in range(8):
                    nc.tensor.matmul(
                        psv,
                        lhsT=st[("xkT", s)][:, dc, jt * 128:(jt + 1) * 128],
                        rhs=self.wv_sb[:, dc, nt * 512:(nt + 1) * 512],
                        start=(dc == 0), stop=(dc == 7),
                    )
                jj = s * (SUP // 128) + jt
                vdst = self.kv[q][1][
                    :, jj, nt * 8 * 65:(nt + 1) * 8 * 65
                ].rearrange("p (h c) -> p h c", c=65)[:, :, 0:64]
                nc.vector.tensor_add(
                    out=vdst,
                    in0=psv.rearrange("p (h c) -> p h c", c=64),
                    in1=self.bv_rep[:, nt * 512:(nt + 1) * 512].rearrange(
                        "p (h c) -> p h c", c=64
                    ),
                )
            return f

        ops = [alloc, load(0)]
        ops += [stats(0, t) for t in range(4)]
        ops.append(rstd(0))
        ops += [norm(0, t) for t in range(4)]
        ops += [tr(0, t) for t in range(4)]
        ops.append(load(1))
        ops += [stats(1, t) for t in range(4)]
        ops.append(rstd(1))
        ops += [norm(1, t) for t in range(4)]
        # super-0 projections with super-1 transposes spread among them
        mix = [kproj(0, e) for e in range(8)]
        mix += [vproj(0, jt, nt) for jt in range(4) for nt in range(2)]
        tr1 = [tr(1, t) for t in range(4)]
        for i, m in enumerate(mix):
            ops.append(m)
            if i % 4 == 3 and tr1:
                ops.append(tr1.pop(0))
        ops += tr1
        ops += [kproj(1, e) for e in range(8)]
        ops += [vproj(1, jt, nt) for jt in range(4) for nt in range(2)]
        return ops

    # ---------- attention ----------

    def attn_unit(self, q, hc, jjp, psos):
        """One double-chunk: 4 sim MMs, 2 exps, 4 PV MMs.  The row-tiled
        sim pair (rows 0-63 / 64-127) must land in DIFFERENT PSUM banks --
        concurrent row-tiles share a bank's write port otherwise."""
        nc = self.nc
        kT, v = self.kv[q]
        pstp0 = self.ps_st.tile([128, 2, NQ], F32, tag="st", name="pstp0")
        pstp1 = self.ps_st.tile([128, 2, NQ], F32, tag="st", name="pstp1")
        pstps = (pstp0, pstp1)
        for u in range(2):
            jj = jjp * 2 + u
            for par in range(2):
                pb = par * 64
                nc.tensor.matmul(
                    pstps[par][:, u, :],
                    lhsT=kT[pb:pb + 64, hc, jj * 128:(jj + 1) * 128],
                    rhs=self.qT[pb:pb + 64, hc, :],
                    start=True, stop=True,
                )
        ets = []
        for par in range(2):
            et = self.etp.tile([128, 2, NQ], MM_DT, tag="et", name="et")
            nc.scalar.activation(
                out=et, in_=pstps[par],
                func=mybir.ActivationFunctionType.Exp,
            )
            ets.append(et)
        for u in range(2):
            jj = jjp * 2 + u
            for par in range(2):
                h = hc * 2 + par
                nc.tensor.matmul(
                    psos[par],
                    lhsT=v[:, jj, h * 65:(h + 1) * 65],
                    rhs=ets[par][:, u, :],
                    start=(jj == 0), stop=(jj == NJJ - 1),
                )

    def pe_filler(self, n=3):
        """Dependency-free identity matmuls into a scratch PSUM bank.
        Quarter 3 has no projection work left to fill the exp-latency
        bubbles; without these the PE duty cycle drops low enough that
        the HAM clock gate throttles to 1.2GHz and stays there."""
        nc = self.nc
        if getattr(self, "_fill", None) is None:
            # borrow the transpose ring's bank -- no transposes run in q3
            self._fill = self.ps_tr.tile([128, 128], F32, tag="tr",
                                         name="psfill")
        for _ in range(n):
            nc.tensor.matmul(
                self._fill, lhsT=self.identb, rhs=self.identb,
                start=True, stop=True,
            )

    def np_recip(self, hc):
        """Stage 1 of head-pair normalize: 1/den in place (DVE, f32)."""
        self.nc.vector.reciprocal(
            out=self.otacc[64:65, hc * 2:hc * 2 + 2, :],
            in_=self.otacc[64:65, hc * 2:hc * 2 + 2, :],
        )

    def np_scale(self, hc):
        """Stage 2: broadcast 1/den down 64 partitions and scale O."""
        nc = self.nc
        psb = self.ps_st.tile([128, 2, NQ], F32, tag="st")
        for k in range(2):
            h = hc * 2 + k
            nc.tensor.matmul(
                psb[0:64, k, :], lhsT=self.ones_t[64:65, :],
                rhs=self.otacc[64:65, h, :],
                start=True, stop=True,
            )
            nc.vector.tensor_mul(
                out=self.ot_n[:, h, :], in0=self.otacc[0:64, h, :],
                in1=psb[0:64, k, :],
            )

    def np_oproj(self, hc):
        """Stage 3: ic=0 half of the out-projection for the head pair."""
        nc = self.nc
        for k in range(2):
            h = hc * 2 + k
            for ft in range(2):
                nc.tensor.matmul(
                    self.psf[ft],
                    lhsT=self.ot_n[:, h, 0:128],
                    rhs=self.wo_sb[:, h, ft * 512:(ft + 1) * 512],
                    start=(h == 0), stop=(h == 15),
                )

    def attention_ops(self, q):
        nc = self.nc
        ops = []
        st = {}

        def unit(hc, jjp):
            def f():
                if ("psos", hc) not in st:
                    st[("psos", hc)] = [
                        self.ps_ot.tile([65, NQ], F32, tag="ot0", name="psos0"),
                        self.ps_ot.tile([65, NQ], F32, tag="ot1", name="psos1"),
                    ]
                self.attn_unit(q, hc, jjp, st[("psos", hc)])
            return f

        def fin(hc):
            def f():
                psos = st.pop(("psos", hc))
                for par in range(2):
                    dst = self.otacc[:, hc * 2 + par, :]
                    if q == 0:
                        nc.vector.tensor_copy(out=dst, in_=psos[par])
                    else:
                        nc.vector.tensor_add(out=dst, in0=dst, in1=psos[par])
            return f

        if q == 3:
            def alloc_psf():
                self.psf = [
                    self.ps_mm.tile([128, 512], F32, tag="mm", name="psf")
                    for _ in range(2)
                ]
            ops.append(alloc_psf)
        for hc in range(8):
            for jjp in range(NJJ // 2):
                ops.append(unit(hc, jjp))
                # stagger normalize stages so the PE never waits on the
                # DVE reciprocal/scale (each stage trails by >= 2 units)
                if q == 3 and jjp == 2 and hc >= 1:
                    ops.append(lambda hc=hc: self.np_scale(hc - 1))
            ops.append(fin(hc))
            if q == 3:
                ops.append(lambda hc=hc: self.np_recip(hc))
                if hc >= 1:
                    ops.append(lambda hc=hc: self.np_oproj(hc - 1))
        if q == 3:
            ops.append(lambda: self.np_scale(7))
            ops.append(lambda: self.np_oproj(7))
            out = []
            for i, o in enumerate(ops):
                out.append(o)
                if i == 1:
                    # sustained burst at the quarter boundary so the HAM
                    # SHORT window flips the PE back to full clock
                    out.append(lambda: self.pe_filler(32))
                else:
                    out.append(lambda: self.pe_filler(3))
            ops = out
        return ops

    # ---------- one-time pieces ----------

    def weights_dma(self):
        nc = self.nc
        wk_r = self.wk.rearrange("(c p) e -> p c e", p=128)
        wv_r = self.wv.rearrange("(c p) e -> p c e", p=128)
        for dc in range(8):
            nc.scalar.dma_start(out=self.wk_sb[:, dc, :], in_=wk_r[:, dc, :])
        for dc in range(8):
            nc.scalar.dma_start(out=self.wv_sb[:, dc, :], in_=wv_r[:, dc, :])

    def wo_dma(self):
        self.nc.scalar.dma_start(
            out=self.wo_sb, in_=self.wo.rearrange("(h p) f -> p h f", p=64)
        )

    def qproj_ops(self):
        nc = self.nc
        ops = []
        st = {}

        def load():
            qts = []
            for t in range(2):
                qt = self.xpool.tile([128, D], F32, tag="xt")
                nc.sync.dma_start(out=qt, in_=self.qry[t * 128:(t + 1) * 128, :])
                qts.append(qt)
            st["qts"] = qts
        ops.append(load)

        def ln_and_tr():
            qts = st["qts"]
            mv = self.per.tile([128, 2, 2], F32, tag="mv")
            for t in range(2):
                stt = self.per.tile([128, 2, nc.vector.BN_STATS_DIM], F32,
                                    tag="stats")
                for sg in range(2):
                    nc.vector.bn_stats(
                        out=stt[:, sg, :],
                        in_=qts[t][:, sg * 512:(sg + 1) * 512],
                    )
                nc.vector.bn_aggr(out=mv[:, t, :], in_=stt)
            sig = self.per.tile([128, 2], F32, tag="sig")
            nc.scalar.activation(
                out=sig, in_=mv[:, :, 1],
                func=mybir.ActivationFunctionType.Sqrt,
                bias=self.eps_t, scale=1.0,
            )
            r = self.per.tile([128, 2], F32, tag="rstd")
            nc.vector.reciprocal(out=r, in_=sig)
            qnT = self.xkp.tile([128, 8, SUP], MM_DT, tag="xkT")
            st["qnT"] = qnT
            for t in range(2):
                qnb = self.xpool.tile([128, D], MM_DT, tag="xnb")
                nc.vector.tensor_scalar(
                    out=qnb, in0=qts[t], scalar1=mv[:, t, 0:1],
                    scalar2=r[:, t:t + 1],
                    op0=mybir.AluOpType.subtract, op1=mybir.AluOpType.mult,
                )
                for c in range(4):
                    ptr = self.ps_tr.tile([128, 2, 128], MM_DT, tag="tr")
                    for k in range(2):
                        dc = c * 2 + k
                        nc.tensor.transpose(
                            ptr[:, k, :], qnb[:, dc * 128:(dc + 1) * 128],
                            self.identb,
                        )
                    nc.vector.tensor_copy(
                        out=qnT[:, c * 2:c * 2 + 2, t * 128:(t + 1) * 128],
                        in_=ptr,
                    )
        ops.append(ln_and_tr)

        wq_r = self.wq.rearrange("(c p) e -> p c e", p=128)

        def proj_ec(ec):
            def f():
                wq_t = self.wstream.tile([128, 8, 128], MM_DT, tag="wqs")
                nc.sync.dma_start(
                    out=wq_t, in_=wq_r[:, :, ec * 128:(ec + 1) * 128]
                )
                psq = self.ps_mm.tile([128, NQ], F32, tag="mm")
                for dc in range(8):
                    nc.tensor.matmul(
                        psq, lhsT=wq_t[:, dc, :], rhs=st["qnT"][:, dc, 0:NQ],
                        start=(dc == 0), stop=(dc == 7),
                    )
                nc.vector.tensor_scalar(
                    out=self.qT[:, ec, :], in0=psq,
                    scalar1=self.bq_sb[:, ec:ec + 1], scalar2=None,
                    op0=mybir.AluOpType.add,
                )
            return f
        for ec in range(8):
            ops.append(proj_ec(ec))
        return ops

    def dummy_out(self):
        nc = self.nc
        osb = self.outp.tile([128, D], F32, tag="outsb", name="osb")
        nc.vector.memset(osb, 0.0)
        nc.sync.dma_start(out=self.out_d[0:128, :], in_=osb)
        nc.sync.dma_start(out=self.out_d[128:256, :], in_=osb)

    def tail(self):
        """ic=0 evac + full ic=1 out-projection chain + store."""
        nc = self.nc
        osb = self.outp.tile([128, D], F32, tag="outsb", name="osb")
        for ft in range(2):
            nc.scalar.activation(
                out=osb[:, ft * 512:(ft + 1) * 512], in_=self.psf[ft],
                func=mybir.ActivationFunctionType.Copy,
            )
        nc.sync.dma_start(out=self.out_d[0:128, :], in_=osb)
        psf2 = [self.ps_mm.tile([128, 512], F32, tag="mm", name="psf2") for _ in range(2)]
        for h in range(16):
            for ft in range(2):
                nc.tensor.matmul(
                    psf2[ft],
                    lhsT=self.ot_n[:, h, 128:256],
                    rhs=self.wo_sb[:, h, ft * 512:(ft + 1) * 512],
                    start=(h == 0), stop=(h == 15),
                )
        osb2 = self.outp.tile([128, D], F32, tag="outsb", name="osb2")
        for ft in range(2):
            nc.scalar.activation(
                out=osb2[:, ft * 512:(ft + 1) * 512], in_=psf2[ft],
                func=mybir.ActivationFunctionType.Copy,
            )
        nc.sync.dma_start(out=self.out_d[128:256, :], in_=osb2)


def _interleave(primary, secondary):
    """Emit all of `primary` with `secondary` spread evenly among them."""
    ops = []
    if not primary:
        return list(secondary)
    ratio = len(secondary) / len(primary)
    acc = 0.0
    si = 0
    for p in primary:
        ops.append(p)
        acc += ratio
        while si < len(secondary) and acc >= 1.0 - 1e-9:
            ops.append(secondary[si])
            si += 1
            acc -= 1.0
    ops.extend(secondary[si:])
    return ops


def _build_body(nc, tc, x, qry, wq, wk, wv, wo, bq, bk, bv, out):
    import contextlib

    ctx = contextlib.ExitStack()
    with ctx:
        b = _Body(nc, tc, ctx, x, qry, wq, wk, wv, wo, bq, bk, bv, out)

        pa0 = b.phaseA_ops(0)
        qp = b.qproj_ops()
        # startup: q load + LN/transpose strictly first (their tiles sit at
        # the head of the shared rings), then quarter-0 LN/transposes with
        # the q-projection matmuls confined to before the super-1 transposes
        # (so the qnT ring slot is provably released in PE order)
        ops = [qp[0], qp[1], pa0[0], pa0[1], b.weights_dma]
        ops += _interleave(pa0[2:24], qp[2:] + [b.wo_dma])
        ops += pa0[24:]
        for o in ops:
            o()

        import os as _os
        seq = bool(int(_os.environ.get("KERNEL_NO_INTERLEAVE", "0")))
        stage = int(_os.environ.get("KERNEL_STAGE", "4"))
        if stage <= 1:
            b.dummy_out()
            return
        for q in range(4):
            attn = b.attention_ops(q)
            nxt = b.phaseA_ops(q + 1) if q < 3 else []
            if seq:
                for o in attn + nxt:
                    o()
                continue
            head, rest = attn[:4], attn[4:]
            for o in head:
                o()
            for o in _interleave(rest, nxt):
                o()
            if stage < 4 and stage <= q + 2:
                b.dummy_out()
                return

        b.tail()


_CACHED = None


def _get_program():
    global _CACHED
    if _CACHED is None:
        _CACHED = build_program()
    return _CACHED


def _prep_inputs(x, query, Wq, Wkv, Wout, ln_q_g, ln_q_b, ln_k_g, ln_k_b):
    scale = DH ** -0.5
    f32 = np.float32
    Wq = np.asarray(Wq, f32)
    Wkv = np.asarray(Wkv, f32)
    Wout = np.asarray(Wout, f32)
    wq_eff = (np.asarray(ln_q_g, f32)[:, None] * Wq * scale).astype(f32)
    bq_eff = (np.asarray(ln_q_b, f32) @ Wq * scale).astype(f32)
    wk_eff = (np.asarray(ln_k_g, f32)[:, None] * Wkv[:, :D]).astype(f32)
    bk_eff = (np.asarray(ln_k_b, f32) @ Wkv[:, :D]).astype(f32)
    wv_eff = (np.asarray(ln_k_g, f32)[:, None] * Wkv[:, D:]).astype(f32)
    bv_eff = (np.asarray(ln_k_b, f32) @ Wkv[:, D:]).astype(f32)
    mdt = _mm_np()
    shared = {
        "qry": np.ascontiguousarray(np.asarray(query, f32)),
        "wq": np.ascontiguousarray(wq_eff.astype(mdt)),
        "wk": np.ascontiguousarray(wk_eff.astype(mdt)),
        "wv": np.ascontiguousarray(wv_eff.astype(mdt)),
        "wo": np.ascontiguousarray(Wout.astype(mdt)),
        "bq": np.ascontiguousarray(bq_eff.reshape(8, 128).T),
        "bk": np.ascontiguousarray(bk_eff.reshape(8, 128).T),
        "bv": np.ascontiguousarray(bv_eff),
    }
    x = np.asarray(x, f32)
    in_maps = [
        dict(shared, x=np.ascontiguousarray(x[i])) for i in range(NCORES)
    ]
    return in_maps


def run(trace=False, **inputs):
    from concourse.bass_utils import run_bass_kernel_spmd

    nc = _get_program()
    in_maps = _prep_inputs(**inputs)
    res = run_bass_kernel_spmd(
        nc, in_maps, core_ids=list(range(NCORES)), trace=trace
    )
    out = np.stack([res.results[i]["out"] for i in range(NCORES)], axis=0)
    return out.astype(np.float32), res.exec_time_ns


def kernel(**inputs):
    out, _ = run(trace=False, **inputs)
    return out


# revision 22
# speedup vs baseline: 1.2224x; 1.0106x over previous
"""AttentionalPooler Trainium2 kernel.

Full inputs -> full outputs; internally data-parallel over batch across 8
NeuronCores (b=8, one batch element per core).

Per-core math (one batch element, all in fp32):
  xk  = LN(x)                      [4096, 1024]
  q   = (LN(query) @ Wq) * scale   [256, 1024]   (identical on every core)
  kT  = Wk'^T @ xk^T               [1024, 4096]  (K stored transposed)
  V   = xk @ Wv'                   [4096, 1024]  (row-major, +ones col/head)
  S^T = kT_h^T-slices @ qT_h       [4096, 256] per head  (j on partitions)
  E   = exp(S^T)  (no max subtraction; |S| <= ~7 so fp32-safe)
  [O^T_h; den_h] = [V_h | 1]^T @ E  accumulated over j   [65, 256]
  out = sum_h (O_h / den_h) @ Wout_h                     [256, 1024]

Schedule: quarters of 1024 keys are software-pipelined -- LN/transpose/
K/V-projection for quarter q+1 (written into double-buffered kT/V tiles)
is interleaved into the attention instruction stream of quarter q, so the
exp (ACT) latency bubbles are filled with projection matmuls and the PE
never idles long enough for the HAM clock gate to re-throttle.  Softmax
normalization and the out-projection are woven into the last quarter,
one head-pair at a time, leaving only a tiny serial tail.
"""

import os
import sys
import types

for _p in ("/root/.axon_site", "/root/.axon_site/_ro/trn_rl_repo", "/opt/trn_rl_repo"):
    if os.path.isdir(_p) and _p not in sys.path:
        sys.path.append(_p)

# The image's antenv package lacks axon_hooks; shim it with the ctypes-based
# NTFF hook from trn_agent_boot so trace=True works under axon.
try:
    import antenv.axon_hooks  # noqa: F401
except ImportError:
    try:
        import trn_agent_boot.trn_boot as _tb

        _hook = _tb._ntff_profile_via_ctypes("/opt/axon/libaxon_pjrt.so")
    except Exception:
        _hook = None
    _m = types.ModuleType("antenv.axon_hooks")
    _m.get_axon_ntff_profile_hook = lambda: _hook
    sys.modules["antenv.axon_hooks"] = _m

import numpy as np

import concourse.bass as bass
import concourse.tile as tile
from concourse import mybir
from concourse.masks import make_identity

D = 1024          # model dim == ctx dim
NCTX = 4096       # keys per batch element
NQ = 256          # queries
H = 16            # heads
DH = 64           # head dim
NCORES = 8
EPS = 1e-5
QTR = 1024        # keys per pipelined quarter
SUP = 512         # projection super-tile (j)
NJJ = QTR // 128  # 128-key chunks per quarter

F32 = mybir.dt.float32
BF16 = mybir.dt.bfloat16

MM_DT = BF16


def _mm_np():
    if MM_DT == F32:
        return np.float32
    import ml_dtypes

    return ml_dtypes.bfloat16


def _patch_drain(max_waits=1):
    """This walrus build rejects >1 sync-wait on the SP Drain that Tile emits
    at kernel exit. Split the waits across a chain of drains."""

    def patched(self, tick_clock, wait_clock):
        from concourse.vector_clock import ScopedClock

        drain_inst = self.nc.sync.drain()
        wait_clock.add_sem_waits(
            drain_inst.ins, ScopedClock({None: tick_clock.global_clock})
        )
        si = drain_inst.ins.sync_info
        waits = list(si.on_wait or []) if si else []
        if len(waits) > max_waits:
            si.on_wait = waits[:max_waits]
            rest = waits[max_waits:]
            while rest:
                extra = self.nc.sync.drain()
                extra.ins.sync_info = mybir.SyncInfo(
                    on_wait=rest[:max_waits], on_update=[]
                )
                rest = rest[max_waits:]
        self.nc.all_engine_barrier()
        assert self.sems is not None
        popped = self.nc._tile_sem_poison_stack.pop()
        assert popped is self._sem_poison
        self.nc.clear_and_free_semaphores(list(self.sems.allocated().values()))
        self.nc.all_engine_barrier()

    tile.TileContext._drain_and_barrier = patched


_patch_drain()


def _split_sync_waits(nc, max_waits=1):
    """This walrus build rejects instructions carrying more than one sync
    wait. Hoist excess waits onto same-engine NoOps placed just before the
    owning instruction (engine queues are serial, so this is equivalent)."""
    for f in nc.m.functions:
        for bb in f.blocks:
            new_list = []
            changed = False
            for inst in bb.instructions:
                si = inst.sync_info
                waits = list(si.on_wait) if si and si.on_wait else []
                if len(waits) > max_waits:
                    changed = True
                    keep = waits[-max_waits:]
                    rest = waits[:-max_waits]
                    k = 0
                    while rest:
                        carrier = mybir.InstNoOp(
                            name=f"{inst.name}-w{k}", ins=[], outs=[]
                        )
                        carrier.engine = inst.engine
                        carrier.sync_info = mybir.SyncInfo(
                            on_wait=rest[:max_waits], on_update=[]
                        )
                        rest = rest[max_waits:]
                        k += 1
                        nc.register_instruction(carrier, overwrite=True)
                        new_list.append(carrier)
                    si.on_wait = keep
                new_list.append(inst)
            if changed:
                bb.instructions = new_list
    return nc


def build_program():
    nc = bass.Bass("TRN2", target_bir_lowering=False, debug=False)

    x = nc.dram_tensor("x", [NCTX, D], F32, kind="ExternalInput").ap()
    qry = nc.dram_tensor("qry", [NQ, D], F32, kind="ExternalInput").ap()
    wq = nc.dram_tensor("wq", [D, D], MM_DT, kind="ExternalInput").ap()
    wk = nc.dram_tensor("wk", [D, D], MM_DT, kind="ExternalInput").ap()
    wv = nc.dram_tensor("wv", [D, D], MM_DT, kind="ExternalInput").ap()
    wo = nc.dram_tensor("wo", [D, D], MM_DT, kind="ExternalInput").ap()
    bq = nc.dram_tensor("bq", [128, 8], F32, kind="ExternalInput").ap()
    bk = nc.dram_tensor("bk", [128, 8], F32, kind="ExternalInput").ap()
    bv = nc.dram_tensor("bv", [D], F32, kind="ExternalInput").ap()
    out = nc.dram_tensor("out", [NQ, D], F32, kind="ExternalOutput").ap()

    with tile.TileContext(nc) as tc:
        _build_body(nc, tc, x, qry, wq, wk, wv, wo, bq, bk, bv, out)
    _split_sync_waits(nc)
    return nc


class _Body:
    """Holds all tiles/pools; methods emit instruction groups."""

    def __init__(self, nc, tc, ctx, x, qry, wq, wk, wv, wo, bq, bk, bv, out):
        self.nc = nc
        self.tc = tc
        self.x, self.qry = x, qry
        self.wq, self.wk, self.wv, self.wo = wq, wk, wv, wo
        self.bq_d, self.bk_d, self.bv_d, self.out_d = bq, bk, bv, out

        ec = ctx.enter_context
        self.consts = ec(tc.tile_pool(name="consts", bufs=1))
        self.wpool = ec(tc.tile_pool(name="wpool", bufs=1))
        self.wstream = ec(tc.tile_pool(name="wstream", bufs=2))
        self.xpool = ec(tc.tile_pool(name="xpool", bufs=4))
        self.big = ec(tc.tile_pool(name="big", bufs=1))
        self.kvp = ec(tc.tile_pool(name="kvp", bufs=2))
        self.xkp = ec(tc.tile_pool(name="xkp", bufs=2))
        self.per = ec(tc.tile_pool(name="per", bufs=2))
        self.etp = ec(tc.tile_pool(name="etp", bufs=4))
        self.outp = ec(tc.tile_pool(name="outp", bufs=2))
        # PSUM budget (8 banks x 2KB):
        #   mm 2x2KB (proj evac; out-proj ic=0 chains live here in q3)
        #   st 2x2KB (sim S^T pairs; also the den-broadcast in q3)
        #   ot0/ot1 1x2KB each (attention accumulators: one bank per
        #   head so the two heads' accumulation groups never share a bank)
        #   tr 2x[128,2,128]bf16 (transposes)
        self.ps_mm = ec(tc.tile_pool(name="ps_mm", bufs=2, space="PSUM"))
        self.ps_st = ec(tc.tile_pool(name="ps_st", bufs=2, space="PSUM"))
        self.ps_ot = ec(tc.tile_pool(name="ps_ot", bufs=1, space="PSUM"))
        self.ps_tr = ec(tc.tile_pool(name="ps_tr", bufs=2, space="PSUM"))

        self.identb = self.consts.tile([128, 128], MM_DT, tag="identb")
        make_identity(nc, self.identb)
        self._fill0 = self.ps_tr.tile([128, 128], F32, tag="tr",
                                      name="psfill0")
        for _ in range(96):
            nc.tensor.matmul(
                self._fill0, lhsT=self.identb, rhs=self.identb,
                start=True, stop=True,
            )
        self.eps_t = self.consts.tile([128, 1], F32, tag="eps")
        nc.vector.memset(self.eps_t, EPS)
        self.ones_t = self.consts.tile([128, DH], F32, tag="ones_t")
        nc.vector.memset(self.ones_t, 1.0)
        self.bq_sb = self.consts.tile([128, 8], F32, tag="bq")
        nc.gpsimd.dma_start(out=self.bq_sb, in_=self.bq_d)
        self.bk_sb = self.consts.tile([128, 8], F32, tag="bk")
        nc.gpsimd.dma_start(out=self.bk_sb, in_=self.bk_d)
        self.bv_rep = self.consts.tile([128, D], F32, tag="bvrep")
        bv_bcast = bass.AP(
            tensor=self.bv_d.tensor, offset=self.bv_d.offset,
            ap=[[0, 128]] + list(self.bv_d.ap),
        )
        nc.gpsimd.dma_start(out=self.bv_rep, in_=bv_bcast)

        self.qT = self.consts.tile([128, 8, NQ], MM_DT, tag="qT")
        self.wk_sb = self.wpool.tile([128, 8, D], MM_DT, tag="wk")
        self.wv_sb = self.wpool.tile([128, 8, D], MM_DT, tag="wv")
        self.wo_sb = self.wpool.tile([64, H, D], MM_DT, tag="wo")
        self.otacc = self.big.tile([65, H, NQ], F32, tag="ot")
        self.ot_n = self.big.tile([64, H, NQ], MM_DT, tag="otn")

        self.kv = {}     # quarter -> (kT_q, v_q) double-buffered tiles
        self.psf = None  # out-proj ic=0 PSUM chains (allocated at q3 start)

    # ---------- phase A (LN + transpose + K/V projection) ----------

    def phaseA_ops(self, q):
        """Closure list building kT/v for quarter q, finely sliced so it can
        be interleaved into the attention stream of quarter q-1."""
        nc = self.nc
        st = {}

        def alloc():
            kT = self.kvp.tile([128, 8, QTR], MM_DT, tag="kt")
            v = self.kvp.tile([128, NJJ, H * 65], MM_DT, tag="vq")
            self.kv[q] = (kT, v)
            ones = v.rearrange("p j (h c) -> p j h c", c=65)[:, :, :, 64:65]
            nc.vector.memset(ones, 1.0)

        def load(s):
            def f():
                xts = []
                for jt in range(SUP // 128):
                    j0 = q * QTR + s * SUP + jt * 128
                    xt = self.xpool.tile([128, D], F32, tag="xt")
                    nc.sync.dma_start(out=xt, in_=self.x[j0:j0 + 128, :])
                    xts.append(xt)
                st[("xt", s)] = xts
            return f

        def stats(s, t):
            def f():
                if ("mv", s) not in st:
                    st[("mv", s)] = self.per.tile([128, 4, 2], F32, tag="mv", name="mv_s")
                xt = st[("xt", s)][t]
                stt = self.per.tile([128, 2, nc.vector.BN_STATS_DIM], F32,
                                    tag="stats")
                for sg in range(2):
                    nc.vector.bn_stats(
                        out=stt[:, sg, :], in_=xt[:, sg * 512:(sg + 1) * 512]
                    )
                nc.vector.bn_aggr(out=st[("mv", s)][:, t, :], in_=stt)
            return f

        def rstd(s):
            def f():
                sig = self.per.tile([128, 4], F32, tag="sig")
                nc.scalar.activation(
                    out=sig, in_=st[("mv", s)][:, :, 1],
                    func=mybir.ActivationFunctionType.Sqrt,
                    bias=self.eps_t, scale=1.0,
                )
                r = self.per.tile([128, 4], F32, tag="rstd")
                nc.vector.reciprocal(out=r, in_=sig)
                st[("rstd", s)] = r
            return f

        def norm(s, t):
            def f():
                xnb = self.xpool.tile([128, D], MM_DT, tag="xnb")
                nc.vector.tensor_scalar(
                    out=xnb, in0=st[("xt", s)][t],
                    scalar1=st[("mv", s)][:, t, 0:1],
                    scalar2=st[("rstd", s)][:, t:t + 1],
                    op0=mybir.AluOpType.subtract, op1=mybir.AluOpType.mult,
                )
                st.setdefault(("xnb", s), {})[t] = xnb
            return f

        def tr(s, t):
            def f():
                if ("xkT", s) not in st:
                    st[("xkT", s)] = self.xkp.tile([128, 8, SUP], MM_DT,
                                                   tag="xkT", name="xkTs")
                xnb = st[("xnb", s)][t]
                xkT = st[("xkT", s)]
                for c in range(4):
                    ptr = self.ps_tr.tile([128, 2, 128], MM_DT, tag="tr")
                    for k in range(2):
                        dc = c * 2 + k
                        nc.tensor.transpose(
                            ptr[:, k, :], xnb[:, dc * 128:(dc + 1) * 128],
                            self.identb,
                        )
                    nc.vector.tensor_copy(
                        out=xkT[:, c * 2:c * 2 + 2, t * 128:(t + 1) * 128],
                        in_=ptr,
                    )
            return f

        def kproj(s, ec):
            def f():
                psk = self.ps_mm.tile([128, SUP], F32, tag="mm")
                for dc in range(8):
                    nc.tensor.matmul(
                        psk,
                        lhsT=self.wk_sb[:, dc, ec * 128:(ec + 1) * 128],
                        rhs=st[("xkT", s)][:, dc, :],
                        start=(dc == 0), stop=(dc == 7),
                    )
                nc.vector.tensor_scalar(
                    out=self.kv[q][0][:, ec, s * SUP:(s + 1) * SUP], in0=psk,
                    scalar1=self.bk_sb[:, ec:ec + 1], scalar2=None,
                    op0=mybir.AluOpType.add,
                )
            return f

        def vproj(s, jt, nt):
            def f():
                psv = self.ps_mm.tile([128, SUP], F32, tag="mm")
                for dc in range(8):
                    nc.tensor.matmul(
                        psv,
                        lhsT=st[("xkT", s)][:, dc, jt * 128:(jt + 1) * 128],
                        rhs=self.wv_sb[:, dc, nt * 512:(nt + 1) * 512],
                        start=(dc == 0), stop=(dc == 7),
                    )
                jj = s * (SUP // 128) + jt
                vdst = self.kv[q][1][
                    :, jj, nt * 8 * 65:(nt + 1) * 8 * 65
                ].rearrange("p (h c) -> p h c", c=65)[:, :, 0:64]
                nc.vector.tensor_add(
                    out=vdst,
                    in0=psv.rearrange("p (h c) -> p h c", c=64),
                    in1=self.bv_rep[:, nt * 512:(nt + 1) * 512].rearrange(
                        "p (h c) -> p h c", c=64
                    ),
                )
            return f

        ops = [alloc, load(0)]
        ops += [stats(0, t) for t in range(4)]
        ops.append(rstd(0))
        ops += [norm(0, t) for t in range(4)]
        ops += [tr(0, t) for t in range(4)]
        ops.append(load(1))
        ops += [stats(1, t) for t in range(4)]
        ops.append(rstd(1))
        ops += [norm(1, t) for t in range(4)]
        # super-0 projections with super-1 transposes spread among them
        mix = [kproj(0, e) for e in range(8)]
        mix += [vproj(0, jt, nt) for jt in range(4) for nt in range(2)]
        tr1 = [tr(1, t) for t in range(4)]
        for i, m in enumerate(mix):
            ops.append(m)
            if i % 4 == 3 and tr1:
                ops.append(tr1.pop(0))
        ops += tr1
        ops += [kproj(1, e) for e in range(8)]
        ops += [vproj(1, jt, nt) for jt in range(4) for nt in range(2)]
        return ops

    # ---------- attention ----------

    def attn_unit(self, q, hc, jjp, psos):
        """One double-chunk: 4 sim MMs, 2 exps, 4 PV MMs.  The row-tiled
        sim pair (rows 0-63 / 64-127) must land in DIFFERENT PSUM banks --
        concurrent row-tiles share a bank's write port otherwise."""
        nc = self.nc
        kT, v = self.kv[q]
        pstp0 = self.ps_st.tile([128, 2, NQ], F32, tag="st", name="pstp0")
        pstp1 = self.ps_st.tile([128, 2, NQ], F32, tag="st", name="pstp1")
        pstps = (pstp0, pstp1)
        for u in range(2):
            jj = jjp * 2 + u
            for par in range(2):
                pb = par * 64
                nc.tensor.matmul(
                    pstps[par][:, u, :],
                    lhsT=kT[pb:pb + 64, hc, jj * 128:(jj + 1) * 128],
                    rhs=self.qT[pb:pb + 64, hc, :],
                    start=True, stop=True,
                )
        ets = []
        for par in range(2):
            et = self.etp.tile([128, 2, NQ], MM_DT, tag="et", name="et")
            nc.scalar.activation(
                out=et, in_=pstps[par],
                func=mybir.ActivationFunctionType.Exp,
            )
            ets.append(et)
        for u in range(2):
            jj = jjp * 2 + u
            for par in range(2):
                h = hc * 2 + par
                nc.tensor.matmul(
                    psos[par],
                    lhsT=v[:, jj, h * 65:(h + 1) * 65],
                    rhs=ets[par][:, u, :],
                    start=(jj == 0), stop=(jj == NJJ - 1),
                )

    def pe_filler(self, n=3):
        """Dependency-free identity matmuls into a scratch PSUM bank.
        Quarter 3 has no projection work left to fill the exp-latency
        bubbles; without these the PE duty cycle drops low enough that
        the HAM clock gate throttles to 1.2GHz and stays there."""
        nc = self.nc
        if getattr(self, "_fill", None) is None:
            # borrow the transpose ring's bank -- no transposes run in q3
            self._fill = self.ps_tr.tile([128, 128], F32, tag="tr",
                                         name="psfill")
        for _ in range(n):
            nc.tensor.matmul(
                self._fill, lhsT=self.identb, rhs=self.identb,
                start=True, stop=True,
            )

    def np_recip(self, hc):
        """Stage 1 of head-pair normalize: 1/den in place (DVE, f32)."""
        self.nc.vector.reciprocal(
            out=self.otacc[64:65, hc * 2:hc * 2 + 2, :],
            in_=self.otacc[64:65, hc * 2:hc * 2 + 2, :],
        )

    def np_scale(self, hc):
        """Stage 2: broadcast 1/den down 64 partitions and scale O."""
        nc = self.nc
        psb = self.ps_st.tile([128, 2, NQ], F32, tag="st")
        for k in range(2):
            h = hc * 2 + k
            nc.tensor.matmul(
                psb[0:64, k, :], lhsT=self.ones_t[64:65, :],
                rhs=self.otacc[64:65, h, :],
                start=True, stop=True,
            )
            nc.vector.tensor_mul(
                out=self.ot_n[:, h, :], in0=self.otacc[0:64, h, :],
                in1=psb[0:64, k, :],
            )

    def np_oproj(self, hc):
        """Stage 3: ic=0 half of the out-projection for the head pair."""
        nc = self.nc
        for k in range(2):
            h = hc * 2 + k
            for ft in range(2):
                nc.tensor.matmul(
                    self.psf[ft],
                    lhsT=self.ot_n[:, h, 0:128],
                    rhs=self.wo_sb[:, h, ft * 512:(ft + 1) * 512],
                    start=(h == 0), stop=(h == 15),
                )

    def attention_ops(self, q):
        nc = self.nc
        ops = []
        st = {}

        def unit(hc, jjp):
            def f():
                if ("psos", hc) not in st:
                    st[("psos", hc)] = [
                        self.ps_ot.tile([65, NQ], F32, tag="ot0", name="psos0"),
                        self.ps_ot.tile([65, NQ], F32, tag="ot1", name="psos1"),
                    ]
                self.attn_unit(q, hc, jjp, st[("psos", hc)])
            return f

        def fin(hc):
            def f():
                psos = st.pop(("psos", hc))
                for par in range(2):
                    dst = self.otacc[:, hc * 2 + par, :]
                    if q == 0:
                        nc.vector.tensor_copy(out=dst, in_=psos[par])
                    else:
                        nc.vector.tensor_add(out=dst, in0=dst, in1=psos[par])
            return f

        if q == 3:
            def alloc_psf():
                self.psf = [
                    self.ps_mm.tile([128, 512], F32, tag="mm", name="psf")
                    for _ in range(2)
                ]
            ops.append(alloc_psf)
        for hc in range(8):
            for jjp in range(NJJ // 2):
                ops.append(unit(hc, jjp))
                # stagger normalize stages so the PE never waits on the
                # DVE reciprocal/scale (each stage trails by >= 2 units)
                if q == 3 and jjp == 2 and hc >= 1:
                    ops.append(lambda hc=hc: self.np_scale(hc - 1))
            ops.append(fin(hc))
            if q == 3:
                ops.append(lambda hc=hc: self.np_recip(hc))
                if hc >= 1:
                    ops.append(lambda hc=hc: self.np_oproj(hc - 1))
        if q == 3:
            ops.append(lambda: self.np_scale(7))
            ops.append(lambda: self.np_oproj(7))
            out = []
            for i, o in enumerate(ops):
                out.append(o)
                if i in (1, 3, 5):
                    # sustained bursts spanning >2 HAM SHORT windows at the
                    # quarter boundary so the PE flips back to full clock
                    out.append(lambda: self.pe_filler(40))
                else:
                    out.append(lambda: self.pe_filler(3))
            ops = out
        return ops

    # ---------- one-time pieces ----------

    def weights_dma(self):
        nc = self.nc
        wk_r = self.wk.rearrange("(c p) e -> p c e", p=128)
        wv_r = self.wv.rearrange("(c p) e -> p c e", p=128)
        for dc in range(8):
            nc.scalar.dma_start(out=self.wk_sb[:, dc, :], in_=wk_r[:, dc, :])
        for dc in range(8):
            nc.scalar.dma_start(out=self.wv_sb[:, dc, :], in_=wv_r[:, dc, :])

    def wo_dma(self):
        self.nc.scalar.dma_start(
            out=self.wo_sb, in_=self.wo.rearrange("(h p) f -> p h f", p=64)
        )

    def qproj_ops(self):
        nc = self.nc
        ops = []
        st = {}

        def load():
            qts = []
            for t in range(2):
                qt = self.xpool.tile([128, D], F32, tag="xt")
                nc.sync.dma_start(out=qt, in_=self.qry[t * 128:(t + 1) * 128, :])
                qts.append(qt)
            st["qts"] = qts
        ops.append(load)

        def ln_and_tr():
            qts = st["qts"]
            mv = self.per.tile([128, 2, 2], F32, tag="mv")
            for t in range(2):
                stt = self.per.tile([128, 2, nc.vector.BN_STATS_DIM], F32,
                                    tag="stats")
                for sg in range(2):
                    nc.vector.bn_stats(
                        out=stt[:, sg, :],
                        in_=qts[t][:, sg * 512:(sg + 1) * 512],
                    )
                nc.vector.bn_aggr(out=mv[:, t, :], in_=stt)
            sig = self.per.tile([128, 2], F32, tag="sig")
            nc.scalar.activation(
                out=sig, in_=mv[:, :, 1],
                func=mybir.ActivationFunctionType.Sqrt,
                bias=self.eps_t, scale=1.0,
            )
            r = self.per.tile([128, 2], F32, tag="rstd")
            nc.vector.reciprocal(out=r, in_=sig)
            qnT = self.xkp.tile([128, 8, SUP], MM_DT, tag="xkT")
            st["qnT"] = qnT
            for t in range(2):
                qnb = self.xpool.tile([128, D], MM_DT, tag="xnb")
                nc.vector.tensor_scalar(
                    out=qnb, in0=qts[t], scalar1=mv[:, t, 0:1],
                    scalar2=r[:, t:t + 1],
                    op0=mybir.AluOpType.subtract, op1=mybir.AluOpType.mult,
                )
                for c in range(4):
                    ptr = self.ps_tr.tile([128, 2, 128], MM_DT, tag="tr")
                    for k in range(2):
                        dc = c * 2 + k
                        nc.tensor.transpose(
                            ptr[:, k, :], qnb[:, dc * 128:(dc + 1) * 128],
                            self.identb,
                        )
                    nc.vector.tensor_copy(
                        out=qnT[:, c * 2:c * 2 + 2, t * 128:(t + 1) * 128],
                        in_=ptr,
                    )
        ops.append(ln_and_tr)

        wq_r = self.wq.rearrange("(c p) e -> p c e", p=128)

        def proj_ec(ec):
            def f():
                wq_t = self.wstream.tile([128, 8, 128], MM_DT, tag="wqs")
                nc.sync.dma_start(
                    out=wq_t, in_=wq_r[:, :, ec * 128:(ec + 1) * 128]
                )
                psq = self.ps_mm.tile([128, NQ], F32, tag="mm")
                for dc in range(8):
                    nc.tensor.matmul(
                        psq, lhsT=wq_t[:, dc, :], rhs=st["qnT"][:, dc, 0:NQ],
                        start=(dc == 0), stop=(dc == 7),
                    )
                nc.vector.tensor_scalar(
                    out=self.qT[:, ec, :], in0=psq,
                    scalar1=self.bq_sb[:, ec:ec + 1], scalar2=None,
                    op0=mybir.AluOpType.add,
                )
            return f
        for ec in range(8):
            ops.append(proj_ec(ec))
        return ops

    def dummy_out(self):
        nc = self.nc
        osb = self.outp.tile([128, D], F32, tag="outsb", name="osb")
        nc.vector.memset(osb, 0.0)
        nc.sync.dma_start(out=self.out_d[0:128, :], in_=osb)
        nc.sync.dma_start(out=self.out_d[128:256, :], in_=osb)

    def tail(self):
        """ic=0 evac + full ic=1 out-projection chain + store."""
        nc = self.nc
        osb = self.outp.tile([128, D], F32, tag="outsb", name="osb")
        for ft in range(2):
            nc.scalar.activation(
                out=osb[:, ft * 512:(ft + 1) * 512], in_=self.psf[ft],
                func=mybir.ActivationFunctionType.Copy,
            )
        nc.sync.dma_start(out=self.out_d[0:128, :], in_=osb)
        psf2 = [self.ps_mm.tile([128, 512], F32, tag="mm", name="psf2") for _ in range(2)]
        for h in range(16):
            for ft in range(2):
                nc.tensor.matmul(
                    psf2[ft],
                    lhsT=self.ot_n[:, h, 128:256],
                    rhs=self.wo_sb[:, h, ft * 512:(ft + 1) * 512],
                    start=(h == 0), stop=(h == 15),
                )
        osb2 = self.outp.tile([128, D], F32, tag="outsb", name="osb2")
        for ft in range(2):
            nc.scalar.activation(
                out=osb2[:, ft * 512:(ft + 1) * 512], in_=psf2[ft],
                func=mybir.ActivationFunctionType.Copy,
            )
        nc.sync.dma_start(out=self.out_d[128:256, :], in_=osb2)


def _interleave(primary, secondary):
    """Emit all of `primary` with `secondary` spread evenly among them."""
    ops = []
    if not primary:
        return list(secondary)
    ratio = len(secondary) / len(primary)
    acc = 0.0
    si = 0
    for p in primary:
        ops.append(p)
        acc += ratio
        while si < len(secondary) and acc >= 1.0 - 1e-9:
            ops.append(secondary[si])
            si += 1
            acc -= 1.0
    ops.extend(secondary[si:])
    return ops


def _build_body(nc, tc, x, qry, wq, wk, wv, wo, bq, bk, bv, out):
    import contextlib

    ctx = contextlib.ExitStack()
    with ctx:
        b = _Body(nc, tc, ctx, x, qry, wq, wk, wv, wo, bq, bk, bv, out)

        pa0 = b.phaseA_ops(0)
        qp = b.qproj_ops()
        # startup: q load + LN/transpose strictly first (their tiles sit at
        # the head of the shared rings), then quarter-0 LN/transposes with
        # the q-projection matmuls confined to before the super-1 transposes
        # (so the qnT ring slot is provably released in PE order)
        ops = [qp[0], qp[1], pa0[0], pa0[1], b.weights_dma]
        ops += _interleave(pa0[2:24], qp[2:] + [b.wo_dma])
        ops += pa0[24:]
        for o in ops:
            o()

        import os as _os
        seq = bool(int(_os.environ.get("KERNEL_NO_INTERLEAVE", "0")))
        stage = int(_os.environ.get("KERNEL_STAGE", "4"))
        if stage <= 1:
            b.dummy_out()
            return
        for q in range(4):
            attn = b.attention_ops(q)
            nxt = b.phaseA_ops(q + 1) if q < 3 else []
            if seq:
                for o in attn + nxt:
                    o()
                continue
            head, rest = attn[:4], attn[4:]
            for o in head:
                o()
            for o in _interleave(rest, nxt):
                o()
            if stage < 4 and stage <= q + 2:
                b.dummy_out()
                return

        b.tail()


_CACHED = None


def _get_program():
    global _CACHED
    if _CACHED is None:
        _CACHED = build_program()
    return _CACHED


def _prep_inputs(x, query, Wq, Wkv, Wout, ln_q_g, ln_q_b, ln_k_g, ln_k_b):
    scale = DH ** -0.5
    f32 = np.float32
    Wq = np.asarray(Wq, f32)
    Wkv = np.asarray(Wkv, f32)
    Wout = np.asarray(Wout, f32)
    wq_eff = (np.asarray(ln_q_g, f32)[:, None] * Wq * scale).astype(f32)
    bq_eff = (np.asarray(ln_q_b, f32) @ Wq * scale).astype(f32)
    wk_eff = (np.asarray(ln_k_g, f32)[:, None] * Wkv[:, :D]).astype(f32)
    bk_eff = (np.asarray(ln_k_b, f32) @ Wkv[:, :D]).astype(f32)
    wv_eff = (np.asarray(ln_k_g, f32)[:, None] * Wkv[:, D:]).astype(f32)
    bv_eff = (np.asarray(ln_k_b, f32) @ Wkv[:, D:]).astype(f32)
    mdt = _mm_np()
    shared = {
        "qry": np.ascontiguousarray(np.asarray(query, f32)),
        "wq": np.ascontiguousarray(wq_eff.astype(mdt)),
        "wk": np.ascontiguousarray(wk_eff.astype(mdt)),
        "wv": np.ascontiguousarray(wv_eff.astype(mdt)),
        "wo": np.ascontiguousarray(Wout.astype(mdt)),
        "bq": np.ascontiguousarray(bq_eff.reshape(8, 128).T),
        "bk": np.ascontiguousarray(bk_eff.reshape(8, 128).T),
        "bv": np.ascontiguousarray(bv_eff),
    }
    x = np.asarray(x, f32)
    in_maps = [
        dict(shared, x=np.ascontiguousarray(x[i])) for i in range(NCORES)
    ]
    return in_maps


def run(trace=False, **inputs):
    from concourse.bass_utils import run_bass_kernel_spmd

    nc = _get_program()
    in_maps = _prep_inputs(**inputs)
    res = run_bass_kernel_spmd(
        nc, in_maps, core_ids=list(range(NCORES)), trace=trace
    )
    out = np.stack([res.results[i]["out"] for i in range(NCORES)], axis=0)
    return out.astype(np.float32), res.exec_time_ns


def kernel(**inputs):
    out, _ = run(trace=False, **inputs)
    return out
